# revision 57
# baseline (speedup 1.0000x reference)
"""BiLevelRoutingAttention (spiking) Trainium2 Bass kernel.

Sharding: 8 cores = 4 batches x 2 L-halves. Routing (top-k over an 8x8
region-affinity matrix per batch) runs on host; each core receives its
query-half tokens plus a host-gathered x_sel containing, for each of its 4
query windows, the 4 routed key windows' raw-x rows (duplicated as needed) so
the device program is identical across cores (SPMD), with no data-dependent
addressing on device.

Device pipeline per core, per timestep t (LIF recurrence over t):
  qT/kT/vT = W.T @ xT matmuls in single-pass fp32r (per-t scale 2^(t-1)
  folded into the host-scaled x), LIF (TT add / is_ge spike / reset via
  scalar_tensor_tensor) on DVE, spikes stored fp16.
  v spikes PE-transposed to natural [tok, ch] layout. Attention per query
  window: ST = k @ qT (keys on partitions) in single-head PSUM rounds
  (double-buffered), exp on ACT (fp16 out), P@V with V stationary (col-strip
  packing over heads), P@ones for softmax denominators, approx-reciprocal +
  fused (ovo * beta) * rsum normalize into oa. Proj runs transposed
  ([ch_out, tok] = wp.T @ oa) in fp32r with the bias folded into the proj
  LIF as a per-partition scalar; output z ships [C, tok] and the host
  transposes back.
"""

import os
import sys
import numpy as np

for _p in ("/root/.axon_site/_ro/trn_rl_repo", "/opt/trn_rl_repo"):
    if os.path.isdir(_p) and _p not in sys.path:
        sys.path.append(_p)

import concourse.bass as bass
import concourse.mybir as mybir
import concourse.tile as tile
from concourse.vector_clock import ScopedClock

if os.environ.get("BK_LDWOPT", "0") == "1":
    from concourse import bass_utils as _bu
    _orig_gwa = _bu.get_walrus_args

    def _gwa(arch, tmpdir, *, dve_root=None):
        return [a.replace("--enable-ldw-opt=false", "--enable-ldw-opt=true")
                for a in _orig_gwa(arch, tmpdir, dve_root=dve_root)]

    _bu.get_walrus_args = _gwa

# ---------------------------------------------------------------- constants
T, B, L, C = 4, 4, 1024, 256
NUM_HEADS, N_WIN, TOPK = 8, 8, 4
HD = C // NUM_HEADS            # 32
WIN = L // N_WIN               # 128
N_CORES = 8
NLOC = 4                       # query windows per core
SEL = TOPK * WIN               # 512 keys per query window
XS_TOK = NLOC * SEL            # 2048 x_sel rows per core
QTOK = NLOC * WIN              # 512 query tokens per core
TAU_SCALE = float(HD) ** -0.5  # attention scale
F16 = mybir.dt.float16
F32 = mybir.dt.float32
F32R = mybir.dt.float32r


# ------------------------------------------------------- tail-drain patch
def _patched_drain_and_barrier(self, tick_clock, wait_clock):
    nc = self.nc
    drain_inst = nc.sync.drain()
    wait_clock.add_sem_waits(
        drain_inst.ins, ScopedClock({None: tick_clock.global_clock})
    )
    waits = list(drain_inst.ins.sync_info.on_wait)
    if len(waits) > 1:
        drain_inst.ins.sync_info.on_wait = waits[:1]
        lst = nc.cur_bb.bb.instructions
        assert lst[-1] is drain_inst.ins
        lst.pop()
        for w in waits[1:]:
            nop = nc.sync.nop(nofuse=True)
            if nop.ins.sync_info is None:
                nop.ins.sync_info = mybir.SyncInfo(on_wait=[], on_update=[])
            nop.ins.sync_info.on_wait.append(w)
        lst.append(drain_inst.ins)
    nc.all_engine_barrier()
    assert self.sems is not None
    popped = nc._tile_sem_poison_stack.pop()
    assert popped is self._sem_poison
    nc.clear_and_free_semaphores(list(self.sems.allocated().values()))
    nc.all_engine_barrier()


tile.TileContext._drain_and_barrier = _patched_drain_and_barrier


# This walrus build accepts at most 2 sem-waits per instruction; move the
# excess onto same-engine NoOps inserted just before, at the BIR-JSON level.
_MAXW = 1


def _split_excess_waits(bir_bytes):
    import orjson
    d = orjson.loads(bir_bytes)
    cnt = 0
    for fn in d.get("functions", []):
        for blk in fn.get("blocks", []):
            out = []
            for ins in blk.get("instructions", []):
                si = ins.get("sync_info")
                waits = (si or {}).get("on_wait") or []
                if len(waits) > _MAXW:
                    keep = waits[:_MAXW]
                    extra = waits[_MAXW:]
                    for j, w in enumerate(extra):
                        cnt += 1
                        out.append({
                            "debug": ins.get("debug", 0),
                            "engine": ins["engine"],
                            "ins": [], "outs": [],
                            "name": f"{ins['name']}-wsplit{j}",
                            "opcode": "NoOp",
                            "sync_info": {"on_update": [], "on_wait": [w]},
                        })
                    si["on_wait"] = keep
                out.append(ins)
            blk["instructions"] = out
    return orjson.dumps(d), cnt


def _wrap_to_json(nc):
    orig = nc.to_json_bytes

    def patched():
        b, cnt = _split_excess_waits(orig())
        return b

    nc.to_json_bytes = patched
    return nc


# ------------------------------------------------------------ host helpers
def _routing_idx(x):
    """Mirror the reference's region routing; x [T,B,L,C] fp32."""
    xs = x.sum(axis=0).reshape(B, N_WIN, WIN, C)
    region = xs.sum(axis=2)                                   # [B, 8, C]
    attn_r = np.einsum("bnc,bmc->bnm", region, region) * (C ** -0.5)
    # top-k (descending, stable) == jax.lax.top_k
    idx = np.argsort(-attn_r, axis=-1, kind="stable")[..., :TOPK]
    return idx.astype(np.int32)                               # [B, 8, 4]


# ------------------------------------------------------------- the program
def _build_program(reps=1):
    nc = bass.Bass("TRN2", target_bir_lowering=False, debug=False,
                   enable_asserts=True, num_devices=N_CORES)

    def din(name, shape, dt=F32):
        return nc.dram_tensor(name, shape, dt, kind="ExternalInput")

    xq = din("xq", [T, C, QTOK], F32R)     # beta_t-scaled on host
    xs = din("xs", [T, C, XS_TOK], F32R)
    wq = din("wq", [C, C], F32R)
    wk = din("wk", [C, C], F32R)
    wv = din("wv", [C, C], F32R)
    wp = din("wp", [C, C], F32R)
    bq = din("bq", [C, T], F32)            # beta_t-scaled on host
    bk = din("bk", [C, T], F32)
    bv = din("bv", [C, T], F32)
    bp = din("bp", [C, T], F32)
    zout = nc.dram_tensor("z", [T, C, QTOK], F32, kind="ExternalOutput")

    KCH = C // 128   # 2 contraction chunks
    G = 2            # channel groups of 128 (4 heads each)
    NKC = XS_TOK // 512  # 4 N-chunks for k/v matmuls (== query windows)

    with tile.TileContext(nc) as tc:
        with (
            tc.tile_pool(name="const", bufs=1) as constp,
            tc.tile_pool(name="xin", bufs=2) as xinp,
            tc.tile_pool(name="state", bufs=1) as statep,
            tc.tile_pool(name="spk", bufs=2) as spkp,
            tc.tile_pool(name="vnat", bufs=2) as vnatp,
            tc.tile_pool(name="pt", bufs=2) as ptp,
            tc.tile_pool(name="oa", bufs=2) as oap,
            tc.tile_pool(name="mm", bufs=2, space="PSUM") as mmp,
            tc.tile_pool(name="st", bufs=2, space="PSUM") as stp,
            tc.tile_pool(name="ov", bufs=1, space="PSUM") as ovp,
        ):
            # constants
            ident = constp.tile([128, 128], F16, tag="ident")
            from concourse.masks import make_identity
            make_identity(nc, ident[:, :])
            ones32 = constp.tile([128, HD], F16, tag="ones32")
            nc.vector.memset(ones32[:, :], 1.0)

            # t-invariant weights (beta folded into x on host)
            wqt = constp.tile([128, KCH * C], F32R, tag="wqt")
            wkt = constp.tile([128, KCH * C], F32R, tag="wkt")
            wvt = constp.tile([128, KCH * C], F32R, tag="wvt")
            wpt = constp.tile([128, KCH * C], F32R, tag="wpt")
            for dst, src in ((wqt, wq), (wkt, wk), (wvt, wv), (wpt, wp)):
                for kc in range(KCH):
                    nc.sync.dma_start(
                        dst[:, kc * C:(kc + 1) * C],
                        src[kc * 128:(kc + 1) * 128, :])

            # all-t biases, loaded once: [128, G*T], col g*T + t
            bqt = constp.tile([128, G * T], F32, tag="bqt")
            bkt = constp.tile([128, G * T], F32, tag="bkt")
            bvt = constp.tile([128, G * T], F32, tag="bvt")
            bpt = constp.tile([128, G * T], F32, tag="bpt")
            for dst, src in ((bqt, bq), (bkt, bk), (bvt, bv), (bpt, bp)):
                for g in range(G):
                    nc.sync.dma_start(
                        dst[:, g * T:(g + 1) * T],
                        src[g * 128:(g + 1) * 128, :])

            # persistent LIF state (u-form), zero-initialized
            uq = statep.tile([128, G * QTOK], F32, tag="uq")
            uk = statep.tile([128, G * XS_TOK], F32, tag="uk")
            uv = statep.tile([128, G * XS_TOK], F32, tag="uv")
            uz = statep.tile([128, G * QTOK], F32, tag="uz")

            def emit_proj(pt, oa_pair):
                """proj + LIF + store for timestep pt (runs one t behind)."""
                th = float(2.0 ** pt)
                for gout in range(G):
                    ps = mmp.tile([128, QTOK], F32, tag="mm")
                    for kc in range(KCH):
                        nc.tensor.matmul(
                            ps[:, :],
                            wpt[:, kc * C + gout * 128:
                                kc * C + (gout + 1) * 128],
                            oa_pair[kc][:, :],
                            start=(kc == 0), stop=(kc == KCH - 1))
                    zs = oap.tile([128, QTOK], F32, tag="zs")
                    u_ap = uz[:, gout * QTOK:(gout + 1) * QTOK]
                    b_ap = bpt[:, gout * T + pt:gout * T + pt + 1]
                    if pt == 0:
                        nc.vector.tensor_scalar(
                            u_ap, ps[:, :], b_ap, None, mybir.AluOpType.add)
                    else:
                        nc.vector.scalar_tensor_tensor(
                            u_ap, ps[:, :], b_ap, u_ap,
                            mybir.AluOpType.add, mybir.AluOpType.add)
                    nc.vector.tensor_scalar(
                        zs[:, :], u_ap, th, None, mybir.AluOpType.is_ge)
                    if pt != T - 1:
                        nc.vector.scalar_tensor_tensor(
                            u_ap, u_ap, th, u_ap,
                            mybir.AluOpType.is_lt, mybir.AluOpType.mult)
                    nc.sync.dma_start(
                        zout[pt, gout * 128:(gout + 1) * 128, :], zs[:, :])

            prev_oa = None
            prev_t = None

            for t_rep in range(reps * T):
                t = t_rep % T
                theta = float(2.0 ** t)
                beta = float(2.0 ** (t - 1))

                # ---- load inputs for this t
                xqh = xinp.tile([128, KCH * QTOK], F32R, tag="xqh")
                xsh = xinp.tile([128, KCH * XS_TOK], F32R, tag="xsh")
                for kc in range(KCH):
                    nc.sync.dma_start(
                        xqh[:, kc * QTOK:(kc + 1) * QTOK],
                        xq[t, kc * 128:(kc + 1) * 128, :])
                    nc.sync.dma_start(
                        xsh[:, kc * XS_TOK:(kc + 1) * XS_TOK],
                        xs[t, kc * 128:(kc + 1) * 128, :])

                # ---- spike tiles for this t
                qs = spkp.tile([128, G * QTOK], F16, tag="qs")
                ks = spkp.tile([128, G * XS_TOK], F16, tag="ks")
                vts = spkp.tile([128, G * XS_TOK], F16, tag="vts")
                # v in natural layout: inst(16) x ch(256)
                vn = vnatp.tile([128, (XS_TOK // 128) * C], F16, tag="vn")

                deferred_resets = []

                def lif_step(u_ap, psum_ap, spike_ap, first, last, bias_ap,
                             defer_reset=False):
                    """u += x + b; s = u >= theta; u = u * (u < theta)."""
                    if first:
                        nc.vector.tensor_scalar(
                            u_ap, psum_ap, bias_ap, None, mybir.AluOpType.add)
                    else:
                        nc.vector.scalar_tensor_tensor(
                            u_ap, psum_ap, bias_ap, u_ap,
                            mybir.AluOpType.add, mybir.AluOpType.add)
                    nc.vector.tensor_scalar(
                        spike_ap, u_ap, theta, None, mybir.AluOpType.is_ge)
                    if not last:
                        def _reset(u_ap=u_ap):
                            nc.vector.scalar_tensor_tensor(
                                u_ap, u_ap, theta, u_ap,
                                mybir.AluOpType.is_lt, mybir.AluOpType.mult)
                        if defer_reset:
                            deferred_resets.append(_reset)
                        else:
                            _reset()

                # ---- qT: [256qch, 512tok] single-pass fp32r
                for g in range(G):
                    ps = mmp.tile([128, QTOK], F32, tag="mm")
                    for kc in range(KCH):
                        nc.tensor.matmul(
                            ps[:, :],
                            wqt[:, kc * C + g * 128: kc * C + (g + 1) * 128],
                            xqh[:, kc * QTOK:(kc + 1) * QTOK],
                            start=(kc == 0), stop=(kc == KCH - 1))
                    lif_step(uq[:, g * QTOK:(g + 1) * QTOK], ps[:, :],
                             qs[:, g * QTOK:(g + 1) * QTOK], t == 0, t == T - 1,
                             bqt[:, g * T + t:g * T + t + 1])

                # ---- kT and vT: [256ch, 2048tok] in 4 N-chunks of 512,
                # k/v interleaved per chunk so spikes for early windows
                # (both tensors) land as soon as possible
                for g in range(G):
                    for nch in range(NKC):
                        for (wt, bias, u_t, s_t) in (
                            (wkt, bkt, uk, ks),
                            (wvt, bvt, uv, vts),
                        ):
                            ps = mmp.tile([128, 512], F32, tag="mm")
                            for kc in range(KCH):
                                nc.tensor.matmul(
                                    ps[:, :],
                                    wt[:, kc * C + g * 128:
                                       kc * C + (g + 1) * 128],
                                    xsh[:, kc * XS_TOK + nch * 512:
                                        kc * XS_TOK + (nch + 1) * 512],
                                    start=(kc == 0), stop=(kc == KCH - 1))
                            off = g * XS_TOK
                            lif_step(u_t[:, off + nch * 512:off + (nch + 1) * 512],
                                     ps[:, :],
                                     s_t[:, off + nch * 512:off + (nch + 1) * 512],
                                     t == 0, t == T - 1,
                                     bias[:, g * T + t:g * T + t + 1],
                                     defer_reset=True)
                            if s_t is vts:
                                # v chunk == window nch: xbar-transpose its
                                # spikes into natural layout right away
                                src = vts[:, off + nch * 512:
                                          off + (nch + 1) * 512]
                                dst = vn[:, :].rearrange(
                                    "p (i c) -> p i c", c=C)[
                                    :, nch * 4:(nch + 1) * 4,
                                    g * 128:(g + 1) * 128]
                                nc.sync.dma_start_transpose(dst, src)

                # ---- proj for the previous timestep (oa(t-1) is ready by
                # now, so these MMs fill the PE while DVE finishes LIF)
                if prev_oa is not None:
                    emit_proj(prev_t, prev_oa)

                # ---- attention per local query window
                oa = []
                for g in range(G):
                    oa_g = oap.tile([128, QTOK], F32R, tag=f"oa{g}", name=f"oa{g}")
                    oa.append(oa_g)
                ovo = [None, None]
                ovs = [None, None]
                rsums = [None, None]
                for n in range(NLOC):
                    nsl = slice(n * 128, (n + 1) * 128)
                    for g in range(G):
                        ptt = ptp.tile([128, 2048], F16, tag="pt")
                        if n == 0:
                            ovo[g] = ovp.tile([128, QTOK], F32, tag=f"ovo{g}", name=f"ovo{g}")
                            ovs[g] = ovp.tile([128, QTOK], F32, tag=f"ovs{g}", name=f"ovs{g}")
                        # S^T: keys on partitions; single-head rounds
                        for h in range(4):
                            stt_ = stp.tile([128, 512], F32, tag="st")
                            for mp in range(4):
                                inst = n * 4 + mp
                                lw = ks[32 * h:32 * (h + 1),
                                        g * XS_TOK + inst * 128:
                                        g * XS_TOK + (inst + 1) * 128]
                                rq = qs[32 * h:32 * (h + 1),
                                        g * QTOK + n * 128:
                                        g * QTOK + (n + 1) * 128]
                                nc.tensor.matmul(
                                    stt_[:, mp * 128:(mp + 1) * 128],
                                    lw, rq, start=True, stop=True,
                                    tile_position=(32 * h, 0))
                            # exp (fp16 out) for this head
                            nc.scalar.activation(
                                ptt[:, h * 512:(h + 1) * 512], stt_[:, :],
                                mybir.ActivationFunctionType.Exp,
                                bias=0.0, scale=TAU_SCALE)
                        # P @ v -> out rows (4 heads stacked on partitions)
                        for h in range(4):
                            hg = g * 4 + h
                            for mp in range(4):
                                inst = n * 4 + mp
                                lv = vn[:, inst * C + hg * HD:
                                        inst * C + (hg + 1) * HD]
                                rp = ptt[:, h * 512 + mp * 128:
                                         h * 512 + (mp + 1) * 128]
                                nc.tensor.matmul(
                                    ovo[g][32 * h:32 * (h + 1), nsl],
                                    lv, rp, start=(mp == 0), stop=(mp == 3),
                                    tile_position=(0, 32 * h))
                        # P @ ones -> softmax denominators (same stacking)
                        for h in range(4):
                            for mp in range(4):
                                rp = ptt[:, h * 512 + mp * 128:
                                         h * 512 + (mp + 1) * 128]
                                nc.tensor.matmul(
                                    ovs[g][32 * h:32 * (h + 1), nsl],
                                    ones32[:, :], rp,
                                    start=(mp == 0), stop=(mp == 3),
                                    tile_position=(0, 32 * h))
                        # drain deferred LIF resets while PE runs attention
                        for _ in range(2):
                            if deferred_resets:
                                deferred_resets.pop(0)()
                        if n == 1 or n == NLOC - 1:
                            # normalize windows {0,1} after n=1, {2,3} after
                            # n=3 (earlier half overlaps remaining windows);
                            # fold beta_t for proj
                            hsl = (slice(0, 256) if n == 1
                                   else slice(256, QTOK))
                            if n == 1:
                                rsums[g] = oap.tile([128, QTOK], F32,
                                                    tag="rsum", name="rsum")
                            sums = oap.tile([128, 256], F32, tag=f"sums{g}")
                            nc.scalar.copy(sums[:, :], ovs[g][:, hsl])
                            nc.vector.reciprocal(rsums[g][:, hsl], sums[:, :])
                            nc.vector.scalar_tensor_tensor(
                                oa[g][:, hsl], ovo[g][:, hsl], beta,
                                rsums[g][:, hsl],
                                mybir.AluOpType.mult, mybir.AluOpType.mult)

                prev_oa = oa
                prev_t = t

            # final timestep's proj
            emit_proj(prev_t, prev_oa)

    return _wrap_to_json(nc)


# ------------------------------------------------------------------ driver
_CACHE = {}


def kernel(x, w_qkv, b_qkv, w_proj, b_proj):
    from concourse.bass_utils import run_bass_kernel_spmd

    x = np.asarray(x, dtype=np.float32)
    w_qkv = np.asarray(w_qkv, dtype=np.float32)
    b_qkv = np.asarray(b_qkv, dtype=np.float32)
    w_proj = np.asarray(w_proj, dtype=np.float32)
    b_proj = np.asarray(b_proj, dtype=np.float32)

    idx = _routing_idx(x)

    # per-t scale (u-form LIF): beta_t = 2^(t-1), folded into x and biases
    betas = np.asarray([2.0 ** (t - 1) for t in range(T)], np.float32)
    wq, wk, wv = w_qkv[:, :C], w_qkv[:, C:2 * C], w_qkv[:, 2 * C:]
    bqv, bkv, bvv = b_qkv[:C], b_qkv[C:2 * C], b_qkv[2 * C:]

    wqT = np.ascontiguousarray(wq)
    wkT = np.ascontiguousarray(wk)
    wvT = np.ascontiguousarray(wv)
    wpT = np.ascontiguousarray(w_proj)
    bq_t = np.ascontiguousarray((betas[None, :] * bqv[:, None]), np.float32)
    bk_t = np.ascontiguousarray((betas[None, :] * bkv[:, None]), np.float32)
    bv_t = np.ascontiguousarray((betas[None, :] * bvv[:, None]), np.float32)
    bp_t = np.ascontiguousarray((betas[None, :] * b_proj[:, None]), np.float32)

    shared = dict(
        wq=wqT, wk=wkT, wv=wvT, wp=wpT,
        bq=bq_t, bk=bk_t, bv=bv_t, bp=bp_t,
    )

    in_maps = []
    for core in range(N_CORES):
        b, half = core // 2, core % 2
        xq = x[:, b, half * QTOK:(half + 1) * QTOK, :]       # [T, 512, C]
        sel_rows = []
        slot_w = []
        for nl in range(NLOC):
            ng = half * NLOC + nl
            for j in range(TOPK):
                w = int(idx[b, ng, j])
                slot_w.append(w)
                sel_rows.append(x[:, b, w * WIN:(w + 1) * WIN, :])
        xsel = np.concatenate(sel_rows, axis=1)               # [T, 2048, C]
        xqT = np.swapaxes(xq, 1, 2) * betas[:, None, None]    # [T, C, 512]
        xsT = np.swapaxes(xsel, 1, 2) * betas[:, None, None]  # [T, C, 2048]
        m = dict(shared)
        m.update(xq=np.ascontiguousarray(xqT, np.float32),
                 xs=np.ascontiguousarray(xsT, np.float32))
        in_maps.append(m)

    key = "prog"
    if key not in _CACHE:
        _CACHE[key] = _build_program()
    nc = _CACHE[key]

    trace = bool(int(os.environ.get("BK_TRACE", "0")))
    res = run_bass_kernel_spmd(
        nc, in_maps, core_ids=list(range(N_CORES)), trace=trace)
    if trace and res.exec_time_ns:
        print(f"HW exec time: {res.exec_time_ns} ns")

    out = np.zeros((T, B, L, C), np.float32)
    for core in range(N_CORES):
        b, half = core // 2, core % 2
        z = res.results[core]["z"]                            # [T, C, 512]
        out[:, b, half * QTOK:(half + 1) * QTOK, :] = np.swapaxes(z, 1, 2)
    return out


# revision 58
# speedup vs baseline: 1.0830x; 1.0830x over previous
"""BiLevelRoutingAttention (spiking) Trainium2 Bass kernel.

Sharding: 8 cores = 4 batches x 2 L-halves. Routing (top-k over an 8x8
region-affinity matrix per batch) runs on host; each core receives its
query-half tokens plus a host-gathered x_sel containing, for each of its 4
query windows, the 4 routed key windows' raw-x rows (duplicated as needed) so
the device program is identical across cores (SPMD), with no data-dependent
addressing on device.

Device pipeline per core, per timestep t (LIF recurrence over t):
  qT/kT/vT = W.T @ xT matmuls in single-pass fp32r (per-t scale 2^(t-1)
  folded into the host-scaled x), LIF (TT add / is_ge spike / reset via
  scalar_tensor_tensor) on DVE, spikes stored fp16.
  v spikes PE-transposed to natural [tok, ch] layout. Attention per query
  window: ST = k @ qT (keys on partitions) in single-head PSUM rounds
  (double-buffered), exp on ACT (fp16 out), P@V with V stationary (col-strip
  packing over heads), P@ones for softmax denominators, approx-reciprocal +
  fused (ovo * beta) * rsum normalize into oa. Proj runs transposed
  ([ch_out, tok] = wp.T @ oa) in fp32r with the bias folded into the proj
  LIF as a per-partition scalar; output z ships [C, tok] and the host
  transposes back.
"""

import os
import sys
import numpy as np

for _p in ("/root/.axon_site/_ro/trn_rl_repo", "/opt/trn_rl_repo"):
    if os.path.isdir(_p) and _p not in sys.path:
        sys.path.append(_p)

import concourse.bass as bass
import concourse.mybir as mybir
import concourse.tile as tile
from concourse.vector_clock import ScopedClock

if os.environ.get("BK_LDWOPT", "0") == "1":
    from concourse import bass_utils as _bu
    _orig_gwa = _bu.get_walrus_args

    def _gwa(arch, tmpdir, *, dve_root=None):
        return [a.replace("--enable-ldw-opt=false", "--enable-ldw-opt=true")
                for a in _orig_gwa(arch, tmpdir, dve_root=dve_root)]

    _bu.get_walrus_args = _gwa

# ---------------------------------------------------------------- constants
T, B, L, C = 4, 4, 1024, 256
NUM_HEADS, N_WIN, TOPK = 8, 8, 4
HD = C // NUM_HEADS            # 32
WIN = L // N_WIN               # 128
N_CORES = 8
NLOC = 4                       # query windows per core
SEL = TOPK * WIN               # 512 keys per query window
XS_TOK = NLOC * SEL            # 2048 x_sel rows per core
QTOK = NLOC * WIN              # 512 query tokens per core
TAU_SCALE = float(HD) ** -0.5  # attention scale
F16 = mybir.dt.float16
F32 = mybir.dt.float32
F32R = mybir.dt.float32r


# ------------------------------------------------------- tail-drain patch
def _patched_drain_and_barrier(self, tick_clock, wait_clock):
    nc = self.nc
    drain_inst = nc.sync.drain()
    wait_clock.add_sem_waits(
        drain_inst.ins, ScopedClock({None: tick_clock.global_clock})
    )
    waits = list(drain_inst.ins.sync_info.on_wait)
    if len(waits) > 1:
        drain_inst.ins.sync_info.on_wait = waits[:1]
        lst = nc.cur_bb.bb.instructions
        assert lst[-1] is drain_inst.ins
        lst.pop()
        for w in waits[1:]:
            nop = nc.sync.nop(nofuse=True)
            if nop.ins.sync_info is None:
                nop.ins.sync_info = mybir.SyncInfo(on_wait=[], on_update=[])
            nop.ins.sync_info.on_wait.append(w)
        lst.append(drain_inst.ins)
    nc.all_engine_barrier()
    assert self.sems is not None
    popped = nc._tile_sem_poison_stack.pop()
    assert popped is self._sem_poison
    nc.clear_and_free_semaphores(list(self.sems.allocated().values()))
    nc.all_engine_barrier()


tile.TileContext._drain_and_barrier = _patched_drain_and_barrier


# This walrus build accepts at most 2 sem-waits per instruction; move the
# excess onto same-engine NoOps inserted just before, at the BIR-JSON level.
_MAXW = 1


def _split_excess_waits(bir_bytes):
    import orjson
    d = orjson.loads(bir_bytes)
    cnt = 0
    for fn in d.get("functions", []):
        for blk in fn.get("blocks", []):
            out = []
            for ins in blk.get("instructions", []):
                si = ins.get("sync_info")
                waits = (si or {}).get("on_wait") or []
                if len(waits) > _MAXW:
                    keep = waits[:_MAXW]
                    extra = waits[_MAXW:]
                    for j, w in enumerate(extra):
                        cnt += 1
                        out.append({
                            "debug": ins.get("debug", 0),
                            "engine": ins["engine"],
                            "ins": [], "outs": [],
                            "name": f"{ins['name']}-wsplit{j}",
                            "opcode": "NoOp",
                            "sync_info": {"on_update": [], "on_wait": [w]},
                        })
                    si["on_wait"] = keep
                out.append(ins)
            blk["instructions"] = out
    return orjson.dumps(d), cnt


def _wrap_to_json(nc):
    orig = nc.to_json_bytes

    def patched():
        b, cnt = _split_excess_waits(orig())
        return b

    nc.to_json_bytes = patched
    return nc


# ------------------------------------------------------------ host helpers
def _routing_idx(x):
    """Mirror the reference's region routing; x [T,B,L,C] fp32."""
    xs = x.sum(axis=0).reshape(B, N_WIN, WIN, C)
    region = xs.sum(axis=2)                                   # [B, 8, C]
    attn_r = np.einsum("bnc,bmc->bnm", region, region) * (C ** -0.5)
    # top-k (descending, stable) == jax.lax.top_k
    idx = np.argsort(-attn_r, axis=-1, kind="stable")[..., :TOPK]
    return idx.astype(np.int32)                               # [B, 8, 4]


# ------------------------------------------------------------- the program
def _build_program(reps=1):
    nc = bass.Bass("TRN2", target_bir_lowering=False, debug=False,
                   enable_asserts=True, num_devices=N_CORES)

    def din(name, shape, dt=F32):
        return nc.dram_tensor(name, shape, dt, kind="ExternalInput")

    xq = din("xq", [T, C, QTOK], F32R)     # beta_t-scaled on host
    xs = din("xs", [T, C, XS_TOK], F32R)
    wq = din("wq", [C, C], F32R)
    wk = din("wk", [C, C], F32R)
    wv = din("wv", [C, C], F32R)
    wp = din("wp", [C, C], F32R)
    bq = din("bq", [C, T], F32)            # beta_t-scaled on host
    bk = din("bk", [C, T], F32)
    bv = din("bv", [C, T], F32)
    bp = din("bp", [C, T], F32)
    zout = nc.dram_tensor("z", [T, C, QTOK], F32, kind="ExternalOutput")

    KCH = C // 128   # 2 contraction chunks
    G = 2            # channel groups of 128 (4 heads each)
    NKC = XS_TOK // 512  # 4 N-chunks for k/v matmuls (== query windows)

    with tile.TileContext(nc) as tc:
        with (
            tc.tile_pool(name="const", bufs=1) as constp,
            tc.tile_pool(name="xin", bufs=2) as xinp,
            tc.tile_pool(name="state", bufs=1) as statep,
            tc.tile_pool(name="spk", bufs=2) as spkp,
            tc.tile_pool(name="vnat", bufs=2) as vnatp,
            tc.tile_pool(name="pt", bufs=2) as ptp,
            tc.tile_pool(name="oa", bufs=2) as oap,
            tc.tile_pool(name="mm", bufs=2, space="PSUM") as mmp,
            tc.tile_pool(name="st", bufs=2, space="PSUM") as stp,
            tc.tile_pool(name="ov", bufs=1, space="PSUM") as ovp,
        ):
            # constants
            ident = constp.tile([128, 128], F16, tag="ident")
            from concourse.masks import make_identity
            make_identity(nc, ident[:, :])
            ones32 = constp.tile([128, HD], F16, tag="ones32")
            nc.vector.memset(ones32[:, :], 1.0)

            # t-invariant weights (beta folded into x on host)
            wqt = constp.tile([128, KCH * C], F32R, tag="wqt")
            wkt = constp.tile([128, KCH * C], F32R, tag="wkt")
            wvt = constp.tile([128, KCH * C], F32R, tag="wvt")
            wpt = constp.tile([128, KCH * C], F32R, tag="wpt")
            for dst, src in ((wqt, wq), (wkt, wk), (wvt, wv), (wpt, wp)):
                for kc in range(KCH):
                    nc.sync.dma_start(
                        dst[:, kc * C:(kc + 1) * C],
                        src[kc * 128:(kc + 1) * 128, :])

            # all-t biases, loaded once: [128, G*T], col g*T + t
            bqt = constp.tile([128, G * T], F32, tag="bqt")
            bkt = constp.tile([128, G * T], F32, tag="bkt")
            bvt = constp.tile([128, G * T], F32, tag="bvt")
            bpt = constp.tile([128, G * T], F32, tag="bpt")
            for dst, src in ((bqt, bq), (bkt, bk), (bvt, bv), (bpt, bp)):
                for g in range(G):
                    nc.sync.dma_start(
                        dst[:, g * T:(g + 1) * T],
                        src[g * 128:(g + 1) * 128, :])

            # persistent LIF state (u-form), zero-initialized
            uq = statep.tile([128, G * QTOK], F32, tag="uq")
            uk = statep.tile([128, G * XS_TOK], F32, tag="uk")
            uv = statep.tile([128, G * XS_TOK], F32, tag="uv")
            uz = statep.tile([128, G * QTOK], F32, tag="uz")

            def emit_proj(pt, oa_pair):
                """proj + LIF + store for timestep pt (runs one t behind)."""
                th = float(2.0 ** pt)
                for gout in range(G):
                    ps = mmp.tile([128, QTOK], F32, tag="mm")
                    for kc in range(KCH):
                        nc.tensor.matmul(
                            ps[:, :],
                            wpt[:, kc * C + gout * 128:
                                kc * C + (gout + 1) * 128],
                            oa_pair[kc][:, :],
                            start=(kc == 0), stop=(kc == KCH - 1))
                    zs = oap.tile([128, QTOK], F32, tag="zs")
                    u_ap = uz[:, gout * QTOK:(gout + 1) * QTOK]
                    b_ap = bpt[:, gout * T + pt:gout * T + pt + 1]
                    if pt == 0:
                        nc.vector.tensor_scalar(
                            u_ap, ps[:, :], b_ap, None, mybir.AluOpType.add)
                    else:
                        nc.vector.scalar_tensor_tensor(
                            u_ap, ps[:, :], b_ap, u_ap,
                            mybir.AluOpType.add, mybir.AluOpType.add)
                    nc.vector.tensor_scalar(
                        zs[:, :], u_ap, th, None, mybir.AluOpType.is_ge)
                    if pt != T - 1:
                        nc.vector.scalar_tensor_tensor(
                            u_ap, u_ap, th, u_ap,
                            mybir.AluOpType.is_lt, mybir.AluOpType.mult)
                    nc.sync.dma_start(
                        zout[pt, gout * 128:(gout + 1) * 128, :], zs[:, :])

            prev_oa = None
            prev_t = None

            for t_rep in range(reps * T):
                t = t_rep % T
                theta = float(2.0 ** t)
                beta = float(2.0 ** (t - 1))

                # ---- load inputs for this t
                xqh = xinp.tile([128, KCH * QTOK], F32R, tag="xqh")
                xsh = xinp.tile([128, KCH * XS_TOK], F32R, tag="xsh")
                for kc in range(KCH):
                    nc.sync.dma_start(
                        xqh[:, kc * QTOK:(kc + 1) * QTOK],
                        xq[t, kc * 128:(kc + 1) * 128, :])
                    nc.sync.dma_start(
                        xsh[:, kc * XS_TOK:(kc + 1) * XS_TOK],
                        xs[t, kc * 128:(kc + 1) * 128, :])

                # ---- spike tiles for this t
                qs = spkp.tile([128, G * QTOK], F16, tag="qs")
                ks = spkp.tile([128, G * XS_TOK], F16, tag="ks")
                vts = spkp.tile([128, G * XS_TOK], F16, tag="vts")
                # v in natural layout: inst(16) x ch(256)
                vn = vnatp.tile([128, (XS_TOK // 128) * C], F16, tag="vn")

                deferred_resets = []

                def lif_step(u_ap, psum_ap, spike_ap, first, last, bias_ap,
                             defer_reset=False):
                    """u += x + b; s = u >= theta; u = u * (u < theta)."""
                    if first:
                        nc.vector.tensor_scalar(
                            u_ap, psum_ap, bias_ap, None, mybir.AluOpType.add)
                    else:
                        nc.vector.scalar_tensor_tensor(
                            u_ap, psum_ap, bias_ap, u_ap,
                            mybir.AluOpType.add, mybir.AluOpType.add)
                    nc.vector.tensor_scalar(
                        spike_ap, u_ap, theta, None, mybir.AluOpType.is_ge)
                    if not last:
                        def _reset(u_ap=u_ap):
                            nc.vector.scalar_tensor_tensor(
                                u_ap, u_ap, theta, u_ap,
                                mybir.AluOpType.is_lt, mybir.AluOpType.mult)
                        if defer_reset:
                            deferred_resets.append(_reset)
                        else:
                            _reset()

                # ---- qT: [256qch, 512tok] single-pass fp32r
                for g in range(G):
                    ps = mmp.tile([128, QTOK], F32, tag="mm")
                    for kc in range(KCH):
                        nc.tensor.matmul(
                            ps[:, :],
                            wqt[:, kc * C + g * 128: kc * C + (g + 1) * 128],
                            xqh[:, kc * QTOK:(kc + 1) * QTOK],
                            start=(kc == 0), stop=(kc == KCH - 1))
                    lif_step(uq[:, g * QTOK:(g + 1) * QTOK], ps[:, :],
                             qs[:, g * QTOK:(g + 1) * QTOK], t == 0, t == T - 1,
                             bqt[:, g * T + t:g * T + t + 1])

                # ---- kT and vT: [256ch, 2048tok] in 4 N-chunks of 512
                for (wt, bias, u_t, s_t) in (
                    (wkt, bkt, uk, ks),
                    (wvt, bvt, uv, vts),
                ):
                    for g in range(G):
                        for nch in range(NKC):
                            ps = mmp.tile([128, 512], F32, tag="mm")
                            for kc in range(KCH):
                                nc.tensor.matmul(
                                    ps[:, :],
                                    wt[:, kc * C + g * 128:
                                       kc * C + (g + 1) * 128],
                                    xsh[:, kc * XS_TOK + nch * 512:
                                        kc * XS_TOK + (nch + 1) * 512],
                                    start=(kc == 0), stop=(kc == KCH - 1))
                            off = g * XS_TOK
                            lif_step(u_t[:, off + nch * 512:off + (nch + 1) * 512],
                                     ps[:, :],
                                     s_t[:, off + nch * 512:off + (nch + 1) * 512],
                                     t == 0, t == T - 1,
                                     bias[:, g * T + t:g * T + t + 1],
                                     defer_reset=True)
                            if s_t is vts:
                                # v chunk == window nch: xbar-transpose its
                                # spikes into natural layout right away
                                src = vts[:, off + nch * 512:
                                          off + (nch + 1) * 512]
                                dst = vn[:, :].rearrange(
                                    "p (i c) -> p i c", c=C)[
                                    :, nch * 4:(nch + 1) * 4,
                                    g * 128:(g + 1) * 128]
                                nc.sync.dma_start_transpose(dst, src)

                # ---- proj for the previous timestep (oa(t-1) is ready by
                # now, so these MMs fill the PE while DVE finishes LIF)
                if prev_oa is not None:
                    emit_proj(prev_t, prev_oa)

                # ---- attention per local query window
                oa = []
                for g in range(G):
                    oa_g = oap.tile([128, QTOK], F32R, tag=f"oa{g}", name=f"oa{g}")
                    oa.append(oa_g)
                ovo = [None, None]
                ovs = [None, None]
                rsums = [None, None]
                for n in range(NLOC):
                    nsl = slice(n * 128, (n + 1) * 128)
                    for g in range(G):
                        ptt = ptp.tile([128, 2048], F16, tag="pt")
                        if n == 0:
                            ovo[g] = ovp.tile([128, QTOK], F32, tag=f"ovo{g}", name=f"ovo{g}")
                            ovs[g] = ovp.tile([128, QTOK], F32, tag=f"ovs{g}", name=f"ovs{g}")
                        # S^T: keys on partitions; single-head rounds
                        for h in range(4):
                            stt_ = stp.tile([128, 512], F32, tag="st")
                            for mp in range(4):
                                inst = n * 4 + mp
                                lw = ks[32 * h:32 * (h + 1),
                                        g * XS_TOK + inst * 128:
                                        g * XS_TOK + (inst + 1) * 128]
                                rq = qs[32 * h:32 * (h + 1),
                                        g * QTOK + n * 128:
                                        g * QTOK + (n + 1) * 128]
                                nc.tensor.matmul(
                                    stt_[:, mp * 128:(mp + 1) * 128],
                                    lw, rq, start=True, stop=True,
                                    tile_position=(32 * h, 0))
                            # exp (fp16 out) for this head
                            nc.scalar.activation(
                                ptt[:, h * 512:(h + 1) * 512], stt_[:, :],
                                mybir.ActivationFunctionType.Exp,
                                bias=0.0, scale=TAU_SCALE)
                        # P @ v -> out rows (4 heads stacked on partitions)
                        for h in range(4):
                            hg = g * 4 + h
                            for mp in range(4):
                                inst = n * 4 + mp
                                lv = vn[:, inst * C + hg * HD:
                                        inst * C + (hg + 1) * HD]
                                rp = ptt[:, h * 512 + mp * 128:
                                         h * 512 + (mp + 1) * 128]
                                nc.tensor.matmul(
                                    ovo[g][32 * h:32 * (h + 1), nsl],
                                    lv, rp, start=(mp == 0), stop=(mp == 3),
                                    tile_position=(0, 32 * h))
                        # P @ ones -> softmax denominators (same stacking)
                        for h in range(4):
                            for mp in range(4):
                                rp = ptt[:, h * 512 + mp * 128:
                                         h * 512 + (mp + 1) * 128]
                                nc.tensor.matmul(
                                    ovs[g][32 * h:32 * (h + 1), nsl],
                                    ones32[:, :], rp,
                                    start=(mp == 0), stop=(mp == 3),
                                    tile_position=(0, 32 * h))
                        # drain deferred LIF resets while PE runs attention
                        for _ in range(2):
                            if deferred_resets:
                                deferred_resets.pop(0)()
                        if n == 1 or n == NLOC - 1:
                            # normalize windows {0,1} after n=1, {2,3} after
                            # n=3 (earlier half overlaps remaining windows);
                            # fold beta_t for proj
                            hsl = (slice(0, 256) if n == 1
                                   else slice(256, QTOK))
                            if n == 1:
                                rsums[g] = oap.tile([128, QTOK], F32,
                                                    tag="rsum", name="rsum")
                            sums = oap.tile([128, 256], F32, tag=f"sums{g}")
                            nc.scalar.copy(sums[:, :], ovs[g][:, hsl])
                            nc.vector.reciprocal(rsums[g][:, hsl], sums[:, :])
                            nc.vector.scalar_tensor_tensor(
                                oa[g][:, hsl], ovo[g][:, hsl], beta,
                                rsums[g][:, hsl],
                                mybir.AluOpType.mult, mybir.AluOpType.mult)

                prev_oa = oa
                prev_t = t

            # final timestep's proj
            emit_proj(prev_t, prev_oa)

    return _wrap_to_json(nc)


# ------------------------------------------------------------------ driver
_CACHE = {}


def kernel(x, w_qkv, b_qkv, w_proj, b_proj):
    from concourse.bass_utils import run_bass_kernel_spmd

    x = np.asarray(x, dtype=np.float32)
    w_qkv = np.asarray(w_qkv, dtype=np.float32)
    b_qkv = np.asarray(b_qkv, dtype=np.float32)
    w_proj = np.asarray(w_proj, dtype=np.float32)
    b_proj = np.asarray(b_proj, dtype=np.float32)

    idx = _routing_idx(x)

    # per-t scale (u-form LIF): beta_t = 2^(t-1), folded into x and biases
    betas = np.asarray([2.0 ** (t - 1) for t in range(T)], np.float32)
    wq, wk, wv = w_qkv[:, :C], w_qkv[:, C:2 * C], w_qkv[:, 2 * C:]
    bqv, bkv, bvv = b_qkv[:C], b_qkv[C:2 * C], b_qkv[2 * C:]

    wqT = np.ascontiguousarray(wq)
    wkT = np.ascontiguousarray(wk)
    wvT = np.ascontiguousarray(wv)
    wpT = np.ascontiguousarray(w_proj)
    bq_t = np.ascontiguousarray((betas[None, :] * bqv[:, None]), np.float32)
    bk_t = np.ascontiguousarray((betas[None, :] * bkv[:, None]), np.float32)
    bv_t = np.ascontiguousarray((betas[None, :] * bvv[:, None]), np.float32)
    bp_t = np.ascontiguousarray((betas[None, :] * b_proj[:, None]), np.float32)

    shared = dict(
        wq=wqT, wk=wkT, wv=wvT, wp=wpT,
        bq=bq_t, bk=bk_t, bv=bv_t, bp=bp_t,
    )

    in_maps = []
    for core in range(N_CORES):
        b, half = core // 2, core % 2
        xq = x[:, b, half * QTOK:(half + 1) * QTOK, :]       # [T, 512, C]
        sel_rows = []
        slot_w = []
        for nl in range(NLOC):
            ng = half * NLOC + nl
            for j in range(TOPK):
                w = int(idx[b, ng, j])
                slot_w.append(w)
                sel_rows.append(x[:, b, w * WIN:(w + 1) * WIN, :])
        xsel = np.concatenate(sel_rows, axis=1)               # [T, 2048, C]
        xqT = np.swapaxes(xq, 1, 2) * betas[:, None, None]    # [T, C, 512]
        xsT = np.swapaxes(xsel, 1, 2) * betas[:, None, None]  # [T, C, 2048]
        m = dict(shared)
        m.update(xq=np.ascontiguousarray(xqT, np.float32),
                 xs=np.ascontiguousarray(xsT, np.float32))
        in_maps.append(m)

    key = "prog"
    if key not in _CACHE:
        _CACHE[key] = _build_program()
    nc = _CACHE[key]

    trace = bool(int(os.environ.get("BK_TRACE", "0")))
    res = run_bass_kernel_spmd(
        nc, in_maps, core_ids=list(range(N_CORES)), trace=trace)
    if trace and res.exec_time_ns:
        print(f"HW exec time: {res.exec_time_ns} ns")

    out = np.zeros((T, B, L, C), np.float32)
    for core in range(N_CORES):
        b, half = core // 2, core % 2
        z = res.results[core]["z"]                            # [T, C, 512]
        out[:, b, half * QTOK:(half + 1) * QTOK, :] = np.swapaxes(z, 1, 2)
    return out
